# revision 18
# baseline (speedup 1.0000x reference)
"""Trainium2 Bass kernel for causal multi-head attention (GPT2-ish block).

Full-input contract: kernel(**inputs) takes the unsharded inputs of
  x: [4, 2048, 768], W_Q/W_K/W_V: [12, 768, 64], W_O: [12, 64, 768],
  b_Q/b_K/b_V: [12, 64], b_O: [768]
and returns out: [4, 2048, 768] fp32.

Sharding: 8 cores = 4 batches x 2 head-groups (6 heads each); each core
computes its batch's attention for its 6 heads through W_O (partial sum over
heads). Host sums the two head-group partials per batch and adds b_O.

Per-core layouts (all matmuls are out = lhsT.T @ rhs on the PE):
  xT   [768, 2048]      (d_model on partitions)
  QT,KT[384, 2048]      (head-features on partitions)  q pre-scaled by 1/8
  V    [2048, 384]      (+ ones column per head for softmax denominators)
  scoresT tiles [sk=128, sq<=512] -> exp on ACT -> causal zero on GPSIMD
  zT'  [65, 512]        rows 0-63 unnormalized z, row 64 = softmax denom
  y    [2048, 768] fp32

Structure notes:
 - Heads are processed in pairs on partition rows 0:64 / 64:128 of one
   feature tile; their K=64 scores matmuls go to different PE row groups and
   run concurrently in the array (measured dstart ~4ns).
 - A pair's two scores tiles share one [128, 2, 512] two-bank psum group so
   a single ACTIVATE exps both heads (amortizes the ~350-cycle ACT setup).
 - Diagonal tiles are trimmed to their valid sq suffix (512-c columns) on
   the PE, ACT, and GPSIMD sides; the causal zeroing runs on GPSIMD.
 - Emission is woven: the independent PE work of proj(q+1) and out(q-1) is
   emitted inside attention chunk q (before each pair's normalize chain) so
   the PE never starves while ACT exps and the HAM clock gate stays at 8/8.
 - The softmax denominator rides as a ones-column in the AV stationary
   ([V | 1]), giving row 64 of zT' = sum(exp); normalization uses the fast
   custom-DVE reciprocal (input staged to partition 0 — the custom op reads
   partition 0 regardless of the AP) and a GPSIMD partition broadcast.
"""

import numpy as np
import ml_dtypes

BF16 = ml_dtypes.bfloat16

S = 2048
D = 768
NH = 12
H = 64
P = 128
NH_LOC = 6
FEAT = NH_LOC * H          # 384
KO = D // P                # 6 contraction tiles for the projections
FT = FEAT // P             # 3 feature partition-tiles
CH = 512                   # sq chunk width
NCH = S // CH              # 4
ST = S // P                # 16 sequence partition-tiles
KPC = CH // P              # 4 sk-tiles per chunk
N_CORES = 8

_NC_CACHE = {}


def _build_nc():
    import concourse.bass as bass
    import concourse.mybir as mybir
    import concourse.tile as tile
    from concourse import bacc

    f32 = mybir.dt.float32
    bf16 = mybir.dt.bfloat16
    AF = mybir.ActivationFunctionType
    ALU = mybir.AluOpType

    nc = bacc.Bacc("TRN2", target_bir_lowering=False, debug=False)

    xT = nc.dram_tensor("xT", [D, S], bf16, kind="ExternalInput").ap()
    wqkv = nc.dram_tensor("wqkv", [D, 3 * FEAT], bf16, kind="ExternalInput").ap()
    bqkv = nc.dram_tensor("bqkv", [3 * FEAT], f32, kind="ExternalInput").ap()
    wo = nc.dram_tensor("wo", [FEAT, D], bf16, kind="ExternalInput").ap()
    y = nc.dram_tensor("y", [S, D], bf16, kind="ExternalOutput").ap()

    with tile.TileContext(nc) as tc:
        with (
            tc.tile_pool(name="persist", bufs=1) as persist,
            tc.tile_pool(name="mm_ps", bufs=2, space="PSUM") as mm_ps,
            tc.tile_pool(name="sc_ps", bufs=2, space="PSUM") as sc_ps,
            tc.tile_pool(name="z_ps", bufs=2, space="PSUM") as z_ps,
            tc.tile_pool(name="p_pool", bufs=10) as p_pool,
            tc.tile_pool(name="n_pool", bufs=6) as n_pool,
            tc.tile_pool(name="y_pool", bufs=3) as y_pool,
        ):
            # ---- load inputs into SBUF ----
            # ordered so the first projection chunk can start ASAP: weights
            # and x chunk-0 land first, interleaved per contraction tile
            x_sb = persist.tile([P, KO, S], bf16)
            xT_t = xT.rearrange("(ko pi) s -> pi ko s", pi=P)
            w_sb = persist.tile([P, KO, 3 * FEAT], bf16)
            w_t = wqkv.rearrange("(ko pi) f -> pi ko f", pi=P)
            # Q weights + x chunk-0 first (unblock the first projection piece),
            # then K weights; V weights and later x chunks follow below
            for ko in range(KO):
                nc.sync.dma_start(out=w_sb[:, ko, 0:FEAT], in_=w_t[:, ko, 0:FEAT])
                nc.scalar.dma_start(out=x_sb[:, ko, 0:CH], in_=xT_t[:, ko, 0:CH])
            for ko in range(KO):
                nc.sync.dma_start(
                    out=w_sb[:, ko, FEAT : 2 * FEAT], in_=w_t[:, ko, FEAT : 2 * FEAT]
                )
            bias_sb = persist.tile([P, 9], f32)
            nc.scalar.dma_start(
                out=bias_sb, in_=bqkv.rearrange("(t pi) -> pi t", pi=P)
            )
            bvb_sb = persist.tile([P, FEAT], f32)
            bv_slice = bqkv[2 * FEAT : 3 * FEAT]
            bv_bcast = bass.AP(
                tensor=bv_slice.tensor,
                offset=bv_slice.offset,
                ap=[[0, P]] + list(bv_slice.ap),
            )
            nc.scalar.dma_start(out=bvb_sb, in_=bv_bcast)
            for ko in range(KO):
                nc.sync.dma_start(
                    out=w_sb[:, ko, 2 * FEAT :], in_=w_t[:, ko, 2 * FEAT :]
                )
            for c in range(1, NCH):
                for ko in range(KO):
                    eng = nc.scalar if (ko % 2) else nc.sync
                    eng.dma_start(
                        out=x_sb[:, ko, c * CH : (c + 1) * CH],
                        in_=xT_t[:, ko, c * CH : (c + 1) * CH],
                    )


            wo_sb = persist.tile([P, FT, D], bf16)
            wo_t = wo.rearrange("(ft pi) d -> pi ft d", pi=P)
            nc.sync.dma_start(out=wo_sb, in_=wo_t)

            qT_sb = persist.tile([P, FT, S], bf16)
            kT_sb = persist.tile([P, FT, S], bf16)
            v_sb = persist.tile([P, ST, NH_LOC, H + 1], bf16)
            zn_sb = persist.tile([P, FT, S], bf16)

            # ones column per head for softmax denominators
            nc.vector.memset(v_sb[:, :, :, H : H + 1], 1.0)

            # preload the exp table on ACT so the first real exp doesn't pay
            # the ~2.7us ACT_TABLE_LOAD in the middle of the pipeline
            warm = n_pool.tile([1, 1], f32, tag="warm")
            nc.vector.memset(warm, 0.0)
            nc.scalar.activation(out=warm, in_=warm, func=AF.Exp)

            # preload the GPSIMD custom-op library too: the first
            # partition_broadcast otherwise triggers a ~6us UNLOAD/LOAD_LIB
            # pair right at the first chunk boundary, stalling every queue
            gw_in = n_pool.tile([1, 4], f32, tag="gwi")
            gw_out = n_pool.tile([2, 4], f32, tag="gwo")
            nc.vector.memset(gw_in, 1.0)
            nc.gpsimd.partition_broadcast(gw_out, gw_in)
            nc.gpsimd.affine_select(
                out=gw_in,
                in_=gw_in,
                compare_op=ALU.is_ge,
                fill=0.0,
                base=0,
                pattern=[[1, 4]],
                channel_multiplier=0,
            )

            # ~14 dummy matmuls sized to finish as the first input DMAs land:
            # they run in the otherwise-idle initial window and push the HAM
            # clock gate to 8/8 before the real projections start
            wrm_sb = persist.tile([P, CH], bf16)
            nc.vector.memset(wrm_sb, 0.0)
            wrm_ps = mm_ps.tile([P, CH], f32, tag="mm", name="wrm_ps")
            for i in range(14):
                nc.tensor.matmul(
                    wrm_ps,
                    lhsT=wrm_sb[:, 0:P],
                    rhs=wrm_sb,
                    start=(i == 0),
                    stop=(i == 13),
                )

            def proj_pieces(c):
                """QT/KT chunk c and V tiles for sequence chunk c, as
                independently emittable pieces."""
                pieces = []
                for which, base in ((0, 0), (1, FEAT)):
                    sb = qT_sb if which == 0 else kT_sb
                    for ft in range(FT):
                        def qk_piece(which=which, base=base, sb=sb, ft=ft):
                            ps = mm_ps.tile([P, CH], f32, tag="mm", name="proj_ps")
                            for ko in range(KO):
                                nc.tensor.matmul(
                                    ps,
                                    lhsT=w_sb[
                                        :, ko, base + ft * P : base + (ft + 1) * P
                                    ],
                                    rhs=x_sb[:, ko, c * CH : (c + 1) * CH],
                                    start=(ko == 0),
                                    stop=(ko == KO - 1),
                                )
                            nc.vector.tensor_scalar_add(
                                out=sb[:, ft, c * CH : (c + 1) * CH],
                                in0=ps,
                                scalar1=bias_sb[
                                    :, which * FT + ft : which * FT + ft + 1
                                ],
                            )
                        pieces.append(qk_piece)
                for sti in range(KPC):
                    def v_piece(sti=sti):
                        st = c * KPC + sti
                        ps = mm_ps.tile([P, FEAT], f32, tag="mm", name="projv_ps")
                        for ko in range(KO):
                            nc.tensor.matmul(
                                ps,
                                lhsT=x_sb[:, ko, st * P : (st + 1) * P],
                                rhs=w_sb[:, ko, 2 * FEAT : 3 * FEAT],
                                start=(ko == 0),
                                stop=(ko == KO - 1),
                            )
                        nc.vector.tensor_add(
                            out=v_sb[:, st, :, 0:H],
                            in0=ps.rearrange("p (n h) -> p n h", h=H),
                            in1=bvb_sb.rearrange("p (n h) -> p n h", h=H),
                        )
                    pieces.append(v_piece)
                return pieces

            def attn_pair(q, hp, fillers=(), tail_warm=False):
                nkt = (q + 1) * KPC
                if True:
                    ft = hp
                    h0, h1 = 2 * hp, 2 * hp + 1
                    zts = [
                        z_ps.tile([H + 1, CH], f32, tag="z", name="zt0"),
                        z_ps.tile([H + 1, CH], f32, tag="z", name="zt1"),
                    ]
                    for kt in range(nkt):
                        c = max(0, kt * P - q * CH)
                        diag = kt >= q * KPC
                        pg = sc_ps.tile([P, 2, CH], f32, tag="sc", name="sc_ps_t")
                        for hi, r0 in ((0, 0), (1, H)):
                            nc.tensor.matmul(
                                pg[:, hi, c:],
                                lhsT=kT_sb[r0 : r0 + H, ft, kt * P : (kt + 1) * P],
                                rhs=qT_sb[r0 : r0 + H, ft, q * CH + c : (q + 1) * CH],
                                start=True,
                                stop=True,
                            )
                        pt = p_pool.tile([P, 2, CH], bf16, tag="pt", name="pt_t")
                        # both heads' (trimmed) tiles exp'd by one ACTIVATE
                        nc.scalar.activation(
                            out=pt[:, :, c:], in_=pg[:, :, c:], func=AF.Exp
                        )
                        if diag:
                            # zero entries with sk > sq for both heads at
                            # once; in the trimmed frame: keep f' - p >= 0
                            nc.gpsimd.affine_select(
                                out=pt[:, :, c:],
                                in_=pt[:, :, c:],
                                compare_op=ALU.is_ge,
                                fill=0.0,
                                base=0,
                                pattern=[[0, 2], [1, CH - c]],
                                channel_multiplier=-1,
                            )
                        for hi, h in ((0, h0), (1, h1)):
                            nc.tensor.matmul(
                                zts[hi][:, c:],
                                lhsT=v_sb[:, kt, h, :],
                                rhs=pt[:, hi, c:],
                                start=(kt == 0),
                                stop=(kt == nkt - 1),
                            )
                    # denominator rows staged straight from psum (row 64 is
                    # final once the last AV stops) so the recip+broadcast
                    # chain starts before the z copies occupy the DVE queue
                    drow2 = n_pool.tile([1, 2, CH], f32, tag="drow", name="drow_t")
                    rrow2 = n_pool.tile([1, 2, CH], f32, tag="rrow", name="rrow_t")
                    rb2 = n_pool.tile([H, 2, CH], f32, tag="rb", name="rb_t")
                    for hi in (0, 1):
                        nc.vector.tensor_copy(
                            out=drow2[:, hi, :], in_=zts[hi][H : H + 1, :]
                        )
                        nc.vector.reciprocal_approx_fast(
                            out=rrow2[:, hi, :], in_=drow2[:, hi, :]
                        )
                        nc.gpsimd.partition_broadcast(rb2[:, hi, :], rrow2[:, hi, :])
                    zsbs = []
                    for hi in (0, 1):
                        zsb = n_pool.tile([H, CH], f32, tag="zsb", name="zsb_t")
                        nc.vector.tensor_copy(out=zsb, in_=zts[hi][0:H, :])
                        zsbs.append(zsb)
                    if tail_warm:
                        # pin a few dummy matmuls to the tail (they consume
                        # the last pt tile, so the scheduler cannot hoist
                        # them): keeps the PE at the warm HAM clock through
                        # the final normalize chain
                        wrm2_ps = mm_ps.tile([P, CH], f32, tag="mm", name="wrm2_ps")
                        for i in range(8):
                            nc.tensor.matmul(
                                wrm2_ps,
                                lhsT=wrm_sb[:, 0:P],
                                rhs=pt[:, 0, :],
                                start=(i == 0),
                                stop=(i == 7),
                            )
                    # independent filler work hides the broadcast latency
                    for f in fillers:
                        f()
                    for hi, r0 in ((0, 0), (1, H)):
                        nc.vector.tensor_mul(
                            out=zn_sb[r0 : r0 + H, ft, q * CH : (q + 1) * CH],
                            in0=zsbs[hi],
                            in1=rb2[:, hi, :],
                        )

            def out_pieces(q):
                # output projection for this chunk's sequence tiles;
                # dh pairs share the stationary zn tile per ft
                pieces = []
                for sti in range(KPC):
                    def out_piece(sti=sti):
                        st = q * KPC + sti
                        ysb = y_pool.tile([P, D], bf16, tag="y", name="y_t")
                        psa = mm_ps.tile([P, D // 2], f32, tag="mm", name="yps_a")
                        psb = mm_ps.tile([P, D // 2], f32, tag="mm", name="yps_b")
                        for ft in range(FT):
                            for ps, dh in ((psa, 0), (psb, 1)):
                                nc.tensor.matmul(
                                    ps,
                                    lhsT=zn_sb[:, ft, st * P : (st + 1) * P],
                                    rhs=wo_sb[
                                        :, ft, dh * (D // 2) : (dh + 1) * (D // 2)
                                    ],
                                    start=(ft == 0),
                                    stop=(ft == FT - 1),
                                )
                        for ps, dh in ((psa, 0), (psb, 1)):
                            nc.vector.tensor_copy(
                                out=ysb[:, dh * (D // 2) : (dh + 1) * (D // 2)],
                                in_=ps,
                            )
                        nc.sync.dma_start(out=y[st * P : (st + 1) * P, :], in_=ysb)
                    pieces.append(out_piece)
                return pieces

            # weave: attention pairs of chunk q are interleaved with the
            # independent PE work of proj(q+1) and out(q-1) so the PE never
            # starves while ACT exps a pair's tiles
            for piece in proj_pieces(0):
                piece()
            for q in range(NCH):
                fillers = []
                if q + 1 < NCH:
                    fillers += proj_pieces(q + 1)
                if q >= 1:
                    fillers += out_pieces(q - 1)
                npairs = NH_LOC // 2
                fi = 0
                for hp in range(npairs):
                    share = ((hp + 1) * len(fillers)) // npairs
                    attn_pair(
                        q,
                        hp,
                        fillers[fi:share],
                        tail_warm=(q == NCH - 1 and hp == npairs - 1),
                    )
                    fi = share
            for piece in out_pieces(NCH - 1):
                piece()

    nc.compile()
    return nc


def _get_nc():
    if "nc" not in _NC_CACHE:
        _NC_CACHE["nc"] = _build_nc()
    return _NC_CACHE["nc"]


def _shard_inputs(x, W_Q, W_K, W_V, W_O, b_Q, b_K, b_V):
    """Build the 8 per-core input maps. Core c -> (batch c % 4, head-group c // 4)."""
    in_maps = []
    scale = np.float32(1.0 / np.sqrt(H))
    for c in range(N_CORES):
        b = c % 4
        g = c // 4
        hs = slice(g * NH_LOC, (g + 1) * NH_LOC)
        xTb = np.ascontiguousarray(x[b].T).astype(BF16)
        wq = (W_Q[hs].transpose(1, 0, 2).reshape(D, FEAT) * scale).astype(BF16)
        wk = W_K[hs].transpose(1, 0, 2).reshape(D, FEAT).astype(BF16)
        wv = W_V[hs].transpose(1, 0, 2).reshape(D, FEAT).astype(BF16)
        wqkv = np.ascontiguousarray(np.concatenate([wq, wk, wv], axis=1))
        bqkv = np.concatenate(
            [
                (b_Q[hs].reshape(FEAT) * scale).astype(np.float32),
                b_K[hs].reshape(FEAT).astype(np.float32),
                b_V[hs].reshape(FEAT).astype(np.float32),
            ]
        )
        wob = np.ascontiguousarray(W_O[hs].reshape(FEAT, D)).astype(BF16)
        in_maps.append({"xT": xTb, "wqkv": wqkv, "bqkv": bqkv, "wo": wob})
    return in_maps


def _run(in_maps, trace=False):
    from concourse.bass_utils import run_bass_kernel_spmd

    nc = _get_nc()
    return run_bass_kernel_spmd(nc, in_maps, core_ids=list(range(N_CORES)), trace=trace)


def kernel(
    normalized_resid_pre,
    W_Q,
    W_K,
    W_V,
    W_O,
    b_Q,
    b_K,
    b_V,
    b_O,
):
    x = np.asarray(normalized_resid_pre, dtype=np.float32)
    in_maps = _shard_inputs(
        x,
        np.asarray(W_Q, np.float32),
        np.asarray(W_K, np.float32),
        np.asarray(W_V, np.float32),
        np.asarray(W_O, np.float32),
        np.asarray(b_Q, np.float32),
        np.asarray(b_K, np.float32),
        np.asarray(b_V, np.float32),
    )
    res = _run(in_maps)
    bO = np.asarray(b_O, np.float32)
    out = np.empty((4, S, D), dtype=np.float32)
    for b in range(4):
        out[b] = (
            res.results[b]["y"].astype(np.float32)
            + res.results[4 + b]["y"].astype(np.float32)
            + bO
        )
    return out



# revision 19
# speedup vs baseline: 1.1132x; 1.1132x over previous
"""Trainium2 Bass kernel for causal multi-head attention (GPT2-ish block).

Full-input contract: kernel(**inputs) takes the unsharded inputs of
  x: [4, 2048, 768], W_Q/W_K/W_V: [12, 768, 64], W_O: [12, 64, 768],
  b_Q/b_K/b_V: [12, 64], b_O: [768]
and returns out: [4, 2048, 768] fp32.

Sharding: 8 cores = 4 batches x 2 head-groups (6 heads each); each core
computes its batch's attention for its 6 heads through W_O (partial sum over
heads). Host sums the two head-group partials per batch and adds b_O.

Per-core layouts (all matmuls are out = lhsT.T @ rhs on the PE):
  xT   [768, 2048]      (d_model on partitions)
  QT,KT[384, 2048]      (head-features on partitions)  q pre-scaled by 1/8
  V    [2048, 384]      (+ ones column per head for softmax denominators)
  scoresT tiles [sk=128, sq<=512] -> exp on ACT -> causal zero on GPSIMD
  zT'  [65, 512]        rows 0-63 unnormalized z, row 64 = softmax denom
  y    [2048, 768] fp32

Structure notes:
 - Heads are processed in pairs on partition rows 0:64 / 64:128 of one
   feature tile; their K=64 scores matmuls go to different PE row groups and
   run concurrently in the array (measured dstart ~4ns).
 - A pair's two scores tiles share one [128, 2, 512] two-bank psum group so
   a single ACTIVATE exps both heads (amortizes the ~350-cycle ACT setup).
 - Diagonal tiles are trimmed to their valid sq suffix (512-c columns) on
   the PE, ACT, and GPSIMD sides; the causal zeroing runs on GPSIMD.
 - Emission is woven: the independent PE work of proj(q+1) and out(q-1) is
   emitted inside attention chunk q (before each pair's normalize chain) so
   the PE never starves while ACT exps and the HAM clock gate stays at 8/8.
 - The softmax denominator rides as a ones-column in the AV stationary
   ([V | 1]), giving row 64 of zT' = sum(exp); normalization uses the fast
   custom-DVE reciprocal (input staged to partition 0 — the custom op reads
   partition 0 regardless of the AP) and a GPSIMD partition broadcast.
"""

import numpy as np
import ml_dtypes

BF16 = ml_dtypes.bfloat16

S = 2048
D = 768
NH = 12
H = 64
P = 128
NH_LOC = 6
FEAT = NH_LOC * H          # 384
KO = D // P                # 6 contraction tiles for the projections
FT = FEAT // P             # 3 feature partition-tiles
CH = 512                   # sq chunk width
NCH = S // CH              # 4
ST = S // P                # 16 sequence partition-tiles
KPC = CH // P              # 4 sk-tiles per chunk
N_CORES = 8

_NC_CACHE = {}


def _build_nc():
    import concourse.bass as bass
    import concourse.mybir as mybir
    import concourse.tile as tile
    from concourse import bacc

    f32 = mybir.dt.float32
    bf16 = mybir.dt.bfloat16
    AF = mybir.ActivationFunctionType
    ALU = mybir.AluOpType

    nc = bacc.Bacc("TRN2", target_bir_lowering=False, debug=False)

    xT = nc.dram_tensor("xT", [D, S], bf16, kind="ExternalInput").ap()
    wqkv = nc.dram_tensor("wqkv", [D, 3 * FEAT], bf16, kind="ExternalInput").ap()
    bqkv = nc.dram_tensor("bqkv", [3 * FEAT], f32, kind="ExternalInput").ap()
    wo = nc.dram_tensor("wo", [FEAT, D], bf16, kind="ExternalInput").ap()
    y = nc.dram_tensor("y", [S, D], bf16, kind="ExternalOutput").ap()

    with tile.TileContext(nc) as tc:
        with (
            tc.tile_pool(name="persist", bufs=1) as persist,
            tc.tile_pool(name="mm_ps", bufs=2, space="PSUM") as mm_ps,
            tc.tile_pool(name="sc_ps", bufs=2, space="PSUM") as sc_ps,
            tc.tile_pool(name="z_ps", bufs=2, space="PSUM") as z_ps,
            tc.tile_pool(name="p_pool", bufs=10) as p_pool,
            tc.tile_pool(name="n_pool", bufs=6) as n_pool,
            tc.tile_pool(name="y_pool", bufs=3) as y_pool,
        ):
            # ---- load inputs into SBUF ----
            # ordered so the first projection chunk can start ASAP: weights
            # and x chunk-0 land first, interleaved per contraction tile
            x_sb = persist.tile([P, KO, S], bf16)
            xT_t = xT.rearrange("(ko pi) s -> pi ko s", pi=P)
            w_sb = persist.tile([P, KO, 3 * FEAT], bf16)
            w_t = wqkv.rearrange("(ko pi) f -> pi ko f", pi=P)
            # Q weights + x chunk-0 first (unblock the first projection piece),
            # then K weights; V weights and later x chunks follow below
            for ko in range(KO):
                nc.sync.dma_start(out=w_sb[:, ko, 0:FEAT], in_=w_t[:, ko, 0:FEAT])
                nc.scalar.dma_start(out=x_sb[:, ko, 0:CH], in_=xT_t[:, ko, 0:CH])
            for ko in range(KO):
                nc.sync.dma_start(
                    out=w_sb[:, ko, FEAT : 2 * FEAT], in_=w_t[:, ko, FEAT : 2 * FEAT]
                )
            bias_sb = persist.tile([P, 9], f32)
            nc.scalar.dma_start(
                out=bias_sb, in_=bqkv.rearrange("(t pi) -> pi t", pi=P)
            )
            bvb_sb = persist.tile([P, FEAT], f32)
            bv_slice = bqkv[2 * FEAT : 3 * FEAT]
            bv_bcast = bass.AP(
                tensor=bv_slice.tensor,
                offset=bv_slice.offset,
                ap=[[0, P]] + list(bv_slice.ap),
            )
            nc.scalar.dma_start(out=bvb_sb, in_=bv_bcast)
            for ko in range(KO):
                nc.sync.dma_start(
                    out=w_sb[:, ko, 2 * FEAT :], in_=w_t[:, ko, 2 * FEAT :]
                )
            for c in range(1, NCH):
                for ko in range(KO):
                    eng = nc.scalar if (ko % 2) else nc.sync
                    eng.dma_start(
                        out=x_sb[:, ko, c * CH : (c + 1) * CH],
                        in_=xT_t[:, ko, c * CH : (c + 1) * CH],
                    )


            wo_sb = persist.tile([P, FT, D], bf16)
            wo_t = wo.rearrange("(ft pi) d -> pi ft d", pi=P)
            nc.sync.dma_start(out=wo_sb, in_=wo_t)

            qT_sb = persist.tile([P, FT, S], bf16)
            kT_sb = persist.tile([P, FT, S], bf16)
            v_sb = persist.tile([P, ST, NH_LOC, H + 1], bf16)
            zn_sb = persist.tile([P, FT, S], bf16)

            # ones column per head for softmax denominators
            nc.vector.memset(v_sb[:, :, :, H : H + 1], 1.0)

            # preload the exp table on ACT so the first real exp doesn't pay
            # the ~2.7us ACT_TABLE_LOAD in the middle of the pipeline
            warm = n_pool.tile([1, 1], f32, tag="warm")
            nc.vector.memset(warm, 0.0)
            nc.scalar.activation(out=warm, in_=warm, func=AF.Exp)

            # (GPSIMD custom-op lib preload removed: an early LOAD_LIB in the
            # NEFF correlated with the whole run clocking at 2.0 instead of
            # 2.4 GHz)

            # ~14 dummy matmuls sized to finish as the first input DMAs land:
            # they run in the otherwise-idle initial window and push the HAM
            # clock gate to 8/8 before the real projections start
            wrm_sb = persist.tile([P, CH], bf16)
            nc.vector.memset(wrm_sb, 0.0)
            wrm_ps = mm_ps.tile([P, CH], f32, tag="mm", name="wrm_ps")
            for i in range(14):
                nc.tensor.matmul(
                    wrm_ps,
                    lhsT=wrm_sb[:, 0:P],
                    rhs=wrm_sb,
                    start=(i == 0),
                    stop=(i == 13),
                )

            def proj_pieces(c):
                """QT/KT chunk c and V tiles for sequence chunk c, as
                independently emittable pieces."""
                pieces = []
                for which, base in ((0, 0), (1, FEAT)):
                    sb = qT_sb if which == 0 else kT_sb
                    for ft in range(FT):
                        def qk_piece(which=which, base=base, sb=sb, ft=ft):
                            ps = mm_ps.tile([P, CH], f32, tag="mm", name="proj_ps")
                            for ko in range(KO):
                                nc.tensor.matmul(
                                    ps,
                                    lhsT=w_sb[
                                        :, ko, base + ft * P : base + (ft + 1) * P
                                    ],
                                    rhs=x_sb[:, ko, c * CH : (c + 1) * CH],
                                    start=(ko == 0),
                                    stop=(ko == KO - 1),
                                )
                            nc.vector.tensor_scalar_add(
                                out=sb[:, ft, c * CH : (c + 1) * CH],
                                in0=ps,
                                scalar1=bias_sb[
                                    :, which * FT + ft : which * FT + ft + 1
                                ],
                            )
                        pieces.append(qk_piece)
                for sti in range(KPC):
                    def v_piece(sti=sti):
                        st = c * KPC + sti
                        ps = mm_ps.tile([P, FEAT], f32, tag="mm", name="projv_ps")
                        for ko in range(KO):
                            nc.tensor.matmul(
                                ps,
                                lhsT=x_sb[:, ko, st * P : (st + 1) * P],
                                rhs=w_sb[:, ko, 2 * FEAT : 3 * FEAT],
                                start=(ko == 0),
                                stop=(ko == KO - 1),
                            )
                        nc.vector.tensor_add(
                            out=v_sb[:, st, :, 0:H],
                            in0=ps.rearrange("p (n h) -> p n h", h=H),
                            in1=bvb_sb.rearrange("p (n h) -> p n h", h=H),
                        )
                    pieces.append(v_piece)
                return pieces

            def attn_pair(q, hp, fillers=(), tail_warm=False):
                nkt = (q + 1) * KPC
                if True:
                    ft = hp
                    h0, h1 = 2 * hp, 2 * hp + 1
                    zts = [
                        z_ps.tile([H + 1, CH], f32, tag="z", name="zt0"),
                        z_ps.tile([H + 1, CH], f32, tag="z", name="zt1"),
                    ]
                    for kt in range(nkt):
                        c = max(0, kt * P - q * CH)
                        diag = kt >= q * KPC
                        pg = sc_ps.tile([P, 2, CH], f32, tag="sc", name="sc_ps_t")
                        for hi, r0 in ((0, 0), (1, H)):
                            nc.tensor.matmul(
                                pg[:, hi, c:],
                                lhsT=kT_sb[r0 : r0 + H, ft, kt * P : (kt + 1) * P],
                                rhs=qT_sb[r0 : r0 + H, ft, q * CH + c : (q + 1) * CH],
                                start=True,
                                stop=True,
                            )
                        pt = p_pool.tile([P, 2, CH], bf16, tag="pt", name="pt_t")
                        # both heads' (trimmed) tiles exp'd by one ACTIVATE
                        nc.scalar.activation(
                            out=pt[:, :, c:], in_=pg[:, :, c:], func=AF.Exp
                        )
                        if diag:
                            # zero entries with sk > sq for both heads at
                            # once; in the trimmed frame: keep f' - p >= 0
                            nc.gpsimd.affine_select(
                                out=pt[:, :, c:],
                                in_=pt[:, :, c:],
                                compare_op=ALU.is_ge,
                                fill=0.0,
                                base=0,
                                pattern=[[0, 2], [1, CH - c]],
                                channel_multiplier=-1,
                            )
                        for hi, h in ((0, h0), (1, h1)):
                            nc.tensor.matmul(
                                zts[hi][:, c:],
                                lhsT=v_sb[:, kt, h, :],
                                rhs=pt[:, hi, c:],
                                start=(kt == 0),
                                stop=(kt == nkt - 1),
                            )
                    # denominator rows staged straight from psum (row 64 is
                    # final once the last AV stops) so the recip+broadcast
                    # chain starts before the z copies occupy the DVE queue
                    drow2 = n_pool.tile([1, 2, CH], f32, tag="drow", name="drow_t")
                    rrow2 = n_pool.tile([1, 2, CH], f32, tag="rrow", name="rrow_t")
                    rb2 = n_pool.tile([H, 2, CH], f32, tag="rb", name="rb_t")
                    for hi in (0, 1):
                        nc.vector.tensor_copy(
                            out=drow2[:, hi, :], in_=zts[hi][H : H + 1, :]
                        )
                        nc.vector.reciprocal_approx_fast(
                            out=rrow2[:, hi, :], in_=drow2[:, hi, :]
                        )
                        nc.gpsimd.partition_broadcast(rb2[:, hi, :], rrow2[:, hi, :])
                    zsbs = []
                    for hi in (0, 1):
                        zsb = n_pool.tile([H, CH], f32, tag="zsb", name="zsb_t")
                        nc.vector.tensor_copy(out=zsb, in_=zts[hi][0:H, :])
                        zsbs.append(zsb)
                    if tail_warm:
                        # pin a few dummy matmuls to the tail (they consume
                        # the last pt tile, so the scheduler cannot hoist
                        # them): keeps the PE at the warm HAM clock through
                        # the final normalize chain
                        wrm2_ps = mm_ps.tile([P, CH], f32, tag="mm", name="wrm2_ps")
                        for i in range(8):
                            nc.tensor.matmul(
                                wrm2_ps,
                                lhsT=wrm_sb[:, 0:P],
                                rhs=pt[:, 0, :],
                                start=(i == 0),
                                stop=(i == 7),
                            )
                    # independent filler work hides the broadcast latency
                    for f in fillers:
                        f()
                    for hi, r0 in ((0, 0), (1, H)):
                        nc.vector.tensor_mul(
                            out=zn_sb[r0 : r0 + H, ft, q * CH : (q + 1) * CH],
                            in0=zsbs[hi],
                            in1=rb2[:, hi, :],
                        )

            def out_pieces(q):
                # output projection for this chunk's sequence tiles;
                # dh pairs share the stationary zn tile per ft
                pieces = []
                for sti in range(KPC):
                    def out_piece(sti=sti):
                        st = q * KPC + sti
                        ysb = y_pool.tile([P, D], bf16, tag="y", name="y_t")
                        psa = mm_ps.tile([P, D // 2], f32, tag="mm", name="yps_a")
                        psb = mm_ps.tile([P, D // 2], f32, tag="mm", name="yps_b")
                        for ft in range(FT):
                            for ps, dh in ((psa, 0), (psb, 1)):
                                nc.tensor.matmul(
                                    ps,
                                    lhsT=zn_sb[:, ft, st * P : (st + 1) * P],
                                    rhs=wo_sb[
                                        :, ft, dh * (D // 2) : (dh + 1) * (D // 2)
                                    ],
                                    start=(ft == 0),
                                    stop=(ft == FT - 1),
                                )
                        for ps, dh in ((psa, 0), (psb, 1)):
                            nc.vector.tensor_copy(
                                out=ysb[:, dh * (D // 2) : (dh + 1) * (D // 2)],
                                in_=ps,
                            )
                        nc.sync.dma_start(out=y[st * P : (st + 1) * P, :], in_=ysb)
                    pieces.append(out_piece)
                return pieces

            # weave: attention pairs of chunk q are interleaved with the
            # independent PE work of proj(q+1) and out(q-1) so the PE never
            # starves while ACT exps a pair's tiles
            for piece in proj_pieces(0):
                piece()
            for q in range(NCH):
                fillers = []
                if q + 1 < NCH:
                    fillers += proj_pieces(q + 1)
                if q >= 1:
                    fillers += out_pieces(q - 1)
                npairs = NH_LOC // 2
                fi = 0
                for hp in range(npairs):
                    share = ((hp + 1) * len(fillers)) // npairs
                    attn_pair(
                        q,
                        hp,
                        fillers[fi:share],
                        tail_warm=(q == NCH - 1 and hp == npairs - 1),
                    )
                    fi = share
            for piece in out_pieces(NCH - 1):
                piece()

    nc.compile()
    return nc


def _get_nc():
    if "nc" not in _NC_CACHE:
        _NC_CACHE["nc"] = _build_nc()
    return _NC_CACHE["nc"]


def _shard_inputs(x, W_Q, W_K, W_V, W_O, b_Q, b_K, b_V):
    """Build the 8 per-core input maps. Core c -> (batch c % 4, head-group c // 4)."""
    in_maps = []
    scale = np.float32(1.0 / np.sqrt(H))
    for c in range(N_CORES):
        b = c % 4
        g = c // 4
        hs = slice(g * NH_LOC, (g + 1) * NH_LOC)
        xTb = np.ascontiguousarray(x[b].T).astype(BF16)
        wq = (W_Q[hs].transpose(1, 0, 2).reshape(D, FEAT) * scale).astype(BF16)
        wk = W_K[hs].transpose(1, 0, 2).reshape(D, FEAT).astype(BF16)
        wv = W_V[hs].transpose(1, 0, 2).reshape(D, FEAT).astype(BF16)
        wqkv = np.ascontiguousarray(np.concatenate([wq, wk, wv], axis=1))
        bqkv = np.concatenate(
            [
                (b_Q[hs].reshape(FEAT) * scale).astype(np.float32),
                b_K[hs].reshape(FEAT).astype(np.float32),
                b_V[hs].reshape(FEAT).astype(np.float32),
            ]
        )
        wob = np.ascontiguousarray(W_O[hs].reshape(FEAT, D)).astype(BF16)
        in_maps.append({"xT": xTb, "wqkv": wqkv, "bqkv": bqkv, "wo": wob})
    return in_maps


def _run(in_maps, trace=False):
    from concourse.bass_utils import run_bass_kernel_spmd

    nc = _get_nc()
    return run_bass_kernel_spmd(nc, in_maps, core_ids=list(range(N_CORES)), trace=trace)


def kernel(
    normalized_resid_pre,
    W_Q,
    W_K,
    W_V,
    W_O,
    b_Q,
    b_K,
    b_V,
    b_O,
):
    x = np.asarray(normalized_resid_pre, dtype=np.float32)
    in_maps = _shard_inputs(
        x,
        np.asarray(W_Q, np.float32),
        np.asarray(W_K, np.float32),
        np.asarray(W_V, np.float32),
        np.asarray(W_O, np.float32),
        np.asarray(b_Q, np.float32),
        np.asarray(b_K, np.float32),
        np.asarray(b_V, np.float32),
    )
    res = _run(in_maps)
    bO = np.asarray(b_O, np.float32)
    out = np.empty((4, S, D), dtype=np.float32)
    for b in range(4):
        out[b] = (
            res.results[b]["y"].astype(np.float32)
            + res.results[4 + b]["y"].astype(np.float32)
            + bO
        )
    return out



# revision 22
# speedup vs baseline: 1.2331x; 1.1077x over previous
"""Trainium2 Bass kernel for causal multi-head attention (GPT2-ish block).

Full-input contract: kernel(**inputs) takes the unsharded inputs of
  x: [4, 2048, 768], W_Q/W_K/W_V: [12, 768, 64], W_O: [12, 64, 768],
  b_Q/b_K/b_V: [12, 64], b_O: [768]
and returns out: [4, 2048, 768] fp32.

Sharding: 8 cores = 4 batches x 2 head-groups (6 heads each); each core
computes its batch's attention for its 6 heads through W_O (partial sum over
heads). Host sums the two head-group partials per batch and adds b_O.

Per-core layouts (all matmuls are out = lhsT.T @ rhs on the PE):
  xT   [768, 2048]      (d_model on partitions)
  QT,KT[384, 2048]      (head-features on partitions)  q pre-scaled by 1/8
  V    [2048, 384]      (+ ones column per head for softmax denominators)
  scoresT tiles [sk=128, sq<=512] -> exp on ACT -> causal zero on GPSIMD
  zT'  [65, 512]        rows 0-63 unnormalized z, row 64 = softmax denom
  y    [2048, 768] fp32

Structure notes:
 - Heads are processed in pairs on partition rows 0:64 / 64:128 of one
   feature tile; their K=64 scores matmuls go to different PE row groups and
   run concurrently in the array (measured dstart ~4ns).
 - A pair's two scores tiles share one [128, 2, 512] two-bank psum group so
   a single ACTIVATE exps both heads (amortizes the ~350-cycle ACT setup).
 - Diagonal tiles are trimmed to their valid sq suffix (512-c columns) on
   the PE, ACT, and GPSIMD sides; the causal zeroing runs on GPSIMD.
 - Emission is woven: the independent PE work of proj(q+1) and out(q-1) is
   emitted inside attention chunk q (before each pair's normalize chain) so
   the PE never starves while ACT exps and the HAM clock gate stays at 8/8.
 - The softmax denominator rides as a ones-column in the AV stationary
   ([V | 1]), giving row 64 of zT' = sum(exp); normalization uses the fast
   custom-DVE reciprocal (input staged to partition 0 — the custom op reads
   partition 0 regardless of the AP) and a GPSIMD partition broadcast.
 - The last pair of the last chunk emits 8 dummy matmuls that consume its pt
   tile: they fill the PE-idle window during the final normalize chain so the
   HAM clock gate stays at 8/8 (2.4 GHz) and the last chunk's output
   projection doesn't run at the cold 1.2 GHz clock.

Perf notes from this session's traces (exec ~212.1us, PE busy ~173us):
 - Engine busy: Tensor ~173us, ACT (exp) ~118-125us, DVE ~105us, GPSIMD
   ~60us (+ ~65us of slow ~0.6us/op semaphore processing on its queue).
 - ACT floor is (free_elems + 352*n_inst)/1.2GHz: ~122us for the 120 exp
   ACTIVATEs — near co-bottleneck with the PE; bigger ACTIVATE batches need
   >8 psum banks, so not reachable at CH=512.
 - The first partition_broadcast triggers a one-time GPSIMD custom-op
   UNLOAD/LOAD_LIB pair (~6us queue stall at the first chunk boundary).
   DO NOT preload it at kernel start: a NEFF with an early LOAD_LIB was
   measured to run the ENTIRE kernel with every engine's clock at 5/6
   (e.g. proj MMs 379ns -> 451-454ns, exec 214.6us -> 259.9-262.8us,
   reproducibly, while an interleaved baseline run stayed at 214.9us).
 - Other measured dead ends (all regressions vs this structure): fusing the
   pair's recip/broadcast into single [1,2,CH] ops (+8us: serializes the two
   heads' normalize chains); bf16 y output + per-head normalize-from-psum
   reorder (+11us combined: perturbs the tile scheduler's weave); SBUF->SBUF
   stride-0-partition broadcast DMA (rejected by the DMA AP verifier);
   ones-column-first V layout (z reads would start at partition 1 — BIR
   requires 32-aligned partition bases).
"""

import numpy as np
import ml_dtypes

BF16 = ml_dtypes.bfloat16

S = 2048
D = 768
NH = 12
H = 64
P = 128
NH_LOC = 6
FEAT = NH_LOC * H          # 384
KO = D // P                # 6 contraction tiles for the projections
FT = FEAT // P             # 3 feature partition-tiles
CH = 512                   # sq chunk width
NCH = S // CH              # 4
ST = S // P                # 16 sequence partition-tiles
KPC = CH // P              # 4 sk-tiles per chunk
N_CORES = 8

_NC_CACHE = {}


def _build_nc():
    import concourse.bass as bass
    import concourse.mybir as mybir
    import concourse.tile as tile
    from concourse import bacc

    f32 = mybir.dt.float32
    bf16 = mybir.dt.bfloat16
    AF = mybir.ActivationFunctionType
    ALU = mybir.AluOpType

    nc = bacc.Bacc("TRN2", target_bir_lowering=False, debug=False)

    xT = nc.dram_tensor("xT", [D, S], bf16, kind="ExternalInput").ap()
    wqkv = nc.dram_tensor("wqkv", [D, 3 * FEAT], bf16, kind="ExternalInput").ap()
    bqkv = nc.dram_tensor("bqkv", [3 * FEAT], f32, kind="ExternalInput").ap()
    wo = nc.dram_tensor("wo", [FEAT, D], bf16, kind="ExternalInput").ap()
    y = nc.dram_tensor("y", [S, D], f32, kind="ExternalOutput").ap()

    with tile.TileContext(nc) as tc:
        with (
            tc.tile_pool(name="persist", bufs=1) as persist,
            tc.tile_pool(name="mm_ps", bufs=2, space="PSUM") as mm_ps,
            tc.tile_pool(name="sc_ps", bufs=2, space="PSUM") as sc_ps,
            tc.tile_pool(name="z_ps", bufs=2, space="PSUM") as z_ps,
            tc.tile_pool(name="p_pool", bufs=10) as p_pool,
            tc.tile_pool(name="n_pool", bufs=6) as n_pool,
            tc.tile_pool(name="y_pool", bufs=3) as y_pool,
        ):
            x_sb = persist.tile([P, KO, S], bf16)
            xT_t = xT.rearrange("(ko pi) s -> pi ko s", pi=P)
            w_sb = persist.tile([P, KO, 3 * FEAT], bf16)
            w_t = wqkv.rearrange("(ko pi) f -> pi ko f", pi=P)
            for ko in range(KO):
                nc.sync.dma_start(
                    out=w_sb[:, ko, 0 : 2 * FEAT], in_=w_t[:, ko, 0 : 2 * FEAT]
                )
                nc.scalar.dma_start(out=x_sb[:, ko, 0:CH], in_=xT_t[:, ko, 0:CH])
            bias_sb = persist.tile([P, 9], f32)
            nc.scalar.dma_start(
                out=bias_sb, in_=bqkv.rearrange("(t pi) -> pi t", pi=P)
            )
            bvb_sb = persist.tile([P, FEAT], f32)
            bv_slice = bqkv[2 * FEAT : 3 * FEAT]
            bv_bcast = bass.AP(
                tensor=bv_slice.tensor,
                offset=bv_slice.offset,
                ap=[[0, P]] + list(bv_slice.ap),
            )
            nc.scalar.dma_start(out=bvb_sb, in_=bv_bcast)
            for ko in range(KO):
                nc.sync.dma_start(
                    out=w_sb[:, ko, 2 * FEAT :], in_=w_t[:, ko, 2 * FEAT :]
                )
            for c in range(1, NCH):
                for ko in range(KO):
                    eng = nc.scalar if (ko % 2) else nc.sync
                    eng.dma_start(
                        out=x_sb[:, ko, c * CH : (c + 1) * CH],
                        in_=xT_t[:, ko, c * CH : (c + 1) * CH],
                    )


            wo_sb = persist.tile([P, FT, D], bf16)
            wo_t = wo.rearrange("(ft pi) d -> pi ft d", pi=P)
            nc.sync.dma_start(out=wo_sb, in_=wo_t)

            qT_sb = persist.tile([P, FT, S], bf16)
            kT_sb = persist.tile([P, FT, S], bf16)
            v_sb = persist.tile([P, ST, NH_LOC, H + 1], bf16)
            zn_sb = persist.tile([P, FT, S], bf16)

            nc.vector.memset(v_sb[:, :, :, H : H + 1], 1.0)

            warm = n_pool.tile([1, 1], f32, tag="warm")
            nc.vector.memset(warm, 0.0)
            nc.scalar.activation(out=warm, in_=warm, func=AF.Exp)

            wrm_sb = persist.tile([P, CH], bf16)
            nc.vector.memset(wrm_sb, 0.0)
            wrm_ps = mm_ps.tile([P, CH], f32, tag="mm", name="wrm_ps")
            for i in range(14):
                nc.tensor.matmul(
                    wrm_ps,
                    lhsT=wrm_sb[:, 0:P],
                    rhs=wrm_sb,
                    start=(i == 0),
                    stop=(i == 13),
                )

            def proj_pieces(c):
                pieces = []
                for which, base in ((0, 0), (1, FEAT)):
                    sb = qT_sb if which == 0 else kT_sb
                    for ft in range(FT):
                        def qk_piece(which=which, base=base, sb=sb, ft=ft):
                            ps = mm_ps.tile([P, CH], f32, tag="mm", name="proj_ps")
                            for ko in range(KO):
                                nc.tensor.matmul(
                                    ps,
                                    lhsT=w_sb[
                                        :, ko, base + ft * P : base + (ft + 1) * P
                                    ],
                                    rhs=x_sb[:, ko, c * CH : (c + 1) * CH],
                                    start=(ko == 0),
                                    stop=(ko == KO - 1),
                                )
                            nc.vector.tensor_scalar_add(
                                out=sb[:, ft, c * CH : (c + 1) * CH],
                                in0=ps,
                                scalar1=bias_sb[
                                    :, which * FT + ft : which * FT + ft + 1
                                ],
                            )
                        pieces.append(qk_piece)
                for sti in range(KPC):
                    def v_piece(sti=sti):
                        st = c * KPC + sti
                        ps = mm_ps.tile([P, FEAT], f32, tag="mm", name="projv_ps")
                        for ko in range(KO):
                            nc.tensor.matmul(
                                ps,
                                lhsT=x_sb[:, ko, st * P : (st + 1) * P],
                                rhs=w_sb[:, ko, 2 * FEAT : 3 * FEAT],
                                start=(ko == 0),
                                stop=(ko == KO - 1),
                            )
                        nc.vector.tensor_add(
                            out=v_sb[:, st, :, 0:H],
                            in0=ps.rearrange("p (n h) -> p n h", h=H),
                            in1=bvb_sb.rearrange("p (n h) -> p n h", h=H),
                        )
                    pieces.append(v_piece)
                return pieces

            def attn_pair(q, hp, fillers=(), tail_warm=False):
                nkt = (q + 1) * KPC
                if True:
                    ft = hp
                    h0, h1 = 2 * hp, 2 * hp + 1
                    zts = [
                        z_ps.tile([H + 1, CH], f32, tag="z", name="zt0"),
                        z_ps.tile([H + 1, CH], f32, tag="z", name="zt1"),
                    ]
                    for kt in range(nkt):
                        c = max(0, kt * P - q * CH)
                        diag = kt >= q * KPC
                        pg = sc_ps.tile([P, 2, CH], f32, tag="sc", name="sc_ps_t")
                        for hi, r0 in ((0, 0), (1, H)):
                            nc.tensor.matmul(
                                pg[:, hi, c:],
                                lhsT=kT_sb[r0 : r0 + H, ft, kt * P : (kt + 1) * P],
                                rhs=qT_sb[r0 : r0 + H, ft, q * CH + c : (q + 1) * CH],
                                start=True,
                                stop=True,
                            )
                        pt = p_pool.tile([P, 2, CH], bf16, tag="pt", name="pt_t")
                        nc.scalar.activation(
                            out=pt[:, :, c:], in_=pg[:, :, c:], func=AF.Exp
                        )
                        if diag:
                            nc.gpsimd.affine_select(
                                out=pt[:, :, c:],
                                in_=pt[:, :, c:],
                                compare_op=ALU.is_ge,
                                fill=0.0,
                                base=0,
                                pattern=[[0, 2], [1, CH - c]],
                                channel_multiplier=-1,
                            )
                        for hi, h in ((0, h0), (1, h1)):
                            nc.tensor.matmul(
                                zts[hi][:, c:],
                                lhsT=v_sb[:, kt, h, :],
                                rhs=pt[:, hi, c:],
                                start=(kt == 0),
                                stop=(kt == nkt - 1),
                            )
                    zsbs = []
                    for hi in (0, 1):
                        zsb = n_pool.tile([H + 1, CH], f32, tag="zsb", name="zsb_t")
                        nc.vector.tensor_copy(out=zsb, in_=zts[hi])
                        zsbs.append(zsb)
                    if tail_warm:
                        # dummy matmuls pinned to the tail (they consume the
                        # last pt tile so the scheduler cannot hoist them):
                        # keep the PE at the warm HAM clock through the final
                        # normalize chain so the last output projection does
                        # not run at the cold 1.2 GHz clock
                        wrm2_ps = mm_ps.tile([P, CH], f32, tag="mm", name="wrm2_ps")
                        for i in range(8):
                            nc.tensor.matmul(
                                wrm2_ps,
                                lhsT=wrm_sb[:, 0:P],
                                rhs=pt[:, 0, :],
                                start=(i == 0),
                                stop=(i == 7),
                            )
                    for f in fillers:
                        f()
                    rbs = []
                    for hi in (0, 1):
                        drow = n_pool.tile([1, CH], f32, tag="drow", name="drow_t")
                        nc.vector.tensor_copy(out=drow, in_=zsbs[hi][H : H + 1, :])
                        rrow = n_pool.tile([1, CH], f32, tag="rrow", name="rrow_t")
                        nc.vector.reciprocal_approx_fast(out=rrow, in_=drow)
                        rb = n_pool.tile([H, CH], f32, tag="rb", name="rb_t")
                        nc.gpsimd.partition_broadcast(rb, rrow)
                        rbs.append(rb)
                    for hi, r0 in ((0, 0), (1, H)):
                        nc.vector.tensor_mul(
                            out=zn_sb[r0 : r0 + H, ft, q * CH : (q + 1) * CH],
                            in0=zsbs[hi][0:H, :],
                            in1=rbs[hi],
                        )

            def out_pieces(q):
                pieces = []
                for sti in range(KPC):
                    def out_piece(sti=sti):
                        st = q * KPC + sti
                        ysb = y_pool.tile([P, D], f32, tag="y", name="y_t")
                        psa = mm_ps.tile([P, D // 2], f32, tag="mm", name="yps_a")
                        psb = mm_ps.tile([P, D // 2], f32, tag="mm", name="yps_b")
                        for ft in range(FT):
                            for ps, dh in ((psa, 0), (psb, 1)):
                                nc.tensor.matmul(
                                    ps,
                                    lhsT=zn_sb[:, ft, st * P : (st + 1) * P],
                                    rhs=wo_sb[
                                        :, ft, dh * (D // 2) : (dh + 1) * (D // 2)
                                    ],
                                    start=(ft == 0),
                                    stop=(ft == FT - 1),
                                )
                        for ps, dh in ((psa, 0), (psb, 1)):
                            nc.vector.tensor_copy(
                                out=ysb[:, dh * (D // 2) : (dh + 1) * (D // 2)],
                                in_=ps,
                            )
                        nc.sync.dma_start(out=y[st * P : (st + 1) * P, :], in_=ysb)
                    pieces.append(out_piece)
                return pieces

            for piece in proj_pieces(0):
                piece()
            for q in range(NCH):
                fillers = []
                if q + 1 < NCH:
                    fillers += proj_pieces(q + 1)
                if q >= 1:
                    fillers += out_pieces(q - 1)
                npairs = NH_LOC // 2
                fi = 0
                for hp in range(npairs):
                    share = ((hp + 1) * len(fillers)) // npairs
                    attn_pair(
                        q,
                        hp,
                        fillers[fi:share],
                        tail_warm=(q == NCH - 1 and hp == npairs - 1),
                    )
                    fi = share
            for piece in out_pieces(NCH - 1):
                piece()

    nc.compile()
    return nc


def _get_nc():
    if "nc" not in _NC_CACHE:
        _NC_CACHE["nc"] = _build_nc()
    return _NC_CACHE["nc"]


def _shard_inputs(x, W_Q, W_K, W_V, W_O, b_Q, b_K, b_V):
    in_maps = []
    scale = np.float32(1.0 / np.sqrt(H))
    for c in range(N_CORES):
        b = c % 4
        g = c // 4
        hs = slice(g * NH_LOC, (g + 1) * NH_LOC)
        xTb = np.ascontiguousarray(x[b].T).astype(BF16)
        wq = (W_Q[hs].transpose(1, 0, 2).reshape(D, FEAT) * scale).astype(BF16)
        wk = W_K[hs].transpose(1, 0, 2).reshape(D, FEAT).astype(BF16)
        wv = W_V[hs].transpose(1, 0, 2).reshape(D, FEAT).astype(BF16)
        wqkv = np.ascontiguousarray(np.concatenate([wq, wk, wv], axis=1))
        bqkv = np.concatenate(
            [
                (b_Q[hs].reshape(FEAT) * scale).astype(np.float32),
                b_K[hs].reshape(FEAT).astype(np.float32),
                b_V[hs].reshape(FEAT).astype(np.float32),
            ]
        )
        wob = np.ascontiguousarray(W_O[hs].reshape(FEAT, D)).astype(BF16)
        in_maps.append({"xT": xTb, "wqkv": wqkv, "bqkv": bqkv, "wo": wob})
    return in_maps


def _run(in_maps, trace=False):
    from concourse.bass_utils import run_bass_kernel_spmd

    nc = _get_nc()
    return run_bass_kernel_spmd(nc, in_maps, core_ids=list(range(N_CORES)), trace=trace)


def kernel(
    normalized_resid_pre,
    W_Q,
    W_K,
    W_V,
    W_O,
    b_Q,
    b_K,
    b_V,
    b_O,
):
    x = np.asarray(normalized_resid_pre, dtype=np.float32)
    in_maps = _shard_inputs(
        x,
        np.asarray(W_Q, np.float32),
        np.asarray(W_K, np.float32),
        np.asarray(W_V, np.float32),
        np.asarray(W_O, np.float32),
        np.asarray(b_Q, np.float32),
        np.asarray(b_K, np.float32),
        np.asarray(b_V, np.float32),
    )
    res = _run(in_maps)
    bO = np.asarray(b_O, np.float32)
    out = np.empty((4, S, D), dtype=np.float32)
    for b in range(4):
        out[b] = res.results[b]["y"] + res.results[4 + b]["y"] + bO
    return out


# revision 24
# speedup vs baseline: 1.2731x; 1.0325x over previous
"""Trainium2 Bass kernel for causal multi-head attention (GPT2-ish block).

Full-input contract: kernel(**inputs) takes the unsharded inputs of
  x: [4, 2048, 768], W_Q/W_K/W_V: [12, 768, 64], W_O: [12, 64, 768],
  b_Q/b_K/b_V: [12, 64], b_O: [768]
and returns out: [4, 2048, 768] fp32.

Sharding: 8 cores = 4 batches x 2 head-groups (6 heads each); each core
computes its batch's attention for its 6 heads through W_O (partial sum over
heads). Host sums the two head-group partials per batch and adds b_O.

Per-core layouts (all matmuls are out = lhsT.T @ rhs on the PE):
  xT   [768, 2048]      (d_model on partitions)
  QT,KT[384, 2048]      (head-features on partitions)  q pre-scaled by 1/8
  V    [2048, 384]      (+ ones column per head for softmax denominators)
  scoresT tiles [sk=128, sq<=512] -> exp on ACT -> causal zero on GPSIMD
  zT'  [65, 512]        rows 0-63 unnormalized z, row 64 = softmax denom
  y    [2048, 768] fp32

Structure notes (see git-less history in the perf notes below):
 - Heads are processed in pairs on partition rows 0:64 / 64:128 of one
   feature tile; their K=64 scores matmuls go to different PE row groups and
   run concurrently in the array (measured dstart ~4ns).
 - A pair's two scores tiles share one [128, 2, 512] two-bank psum group so
   a single ACTIVATE exps both heads (amortizes the ~350-cycle ACT setup).
 - Diagonal tiles are trimmed to their valid sq suffix; causal zeroing on
   GPSIMD (affine_select).
 - Emission is woven: the independent PE work of proj(q+1) and out(q-1) is
   emitted inside attention chunk q so the PE never starves during exps.
 - Softmax denominator rides as a ones-column in the AV stationary ([V | 1]);
   normalization = custom-DVE reciprocal (reads partition 0; denominator row
   staged there) + GPSIMD partition broadcast + DVE multiply.
 - A dummy partition_broadcast right after the ACT exp-table warm triggers
   the GPSIMD custom-op library UNLOAD/LOAD_LIB during the initial DMA-wait
   window; without it the first real broadcast pays a ~6us GPSIMD-queue
   stall at the first chunk boundary that every engine convoys behind
   (measured: 212.1 -> 206.1us).
 - The last pair of the last chunk emits 8 dummy matmuls that consume its pt
   tile (so the scheduler cannot hoist them): they bridge the PE-idle window
   during the final normalize chain so the HAM clock gate stays at 8/8 and
   the last output projection runs at 2.4 GHz, not 1.2 (214.6 -> 212.1us).

Perf notes from trace analysis (best measured: 206124 ns):
 - Engine busy at 206us: Tensor ~171us (largest idle gap 1.3us), ACT (exp)
   ~124us, DVE ~102us, GPSIMD ~62us. ACT floor for the 120 exp ACTIVATEs is
   (free_elems + 352*n)/1.2GHz ~= 122us; batching bigger ACTIVATEs needs >8
   psum banks, unreachable at CH=512.
 - CLOCK TRAP: some NEFF layouts of this kernel run every engine at exactly
   5/6 clock for the whole execution (proj MM 379ns -> ~451ns, exec * 1.20),
   reproducibly for that binary while other builds interleaved on the same
   device stay fast. Observed on: (a) a variant adding a degenerate
   affine_select+broadcast warm-up pair, (b) a variant adding a Q/K-split
   weight-DMA reorder on top of this exact file. The current file measured
   full-clock. If a future edit regresses ~20% uniformly, suspect this
   before blaming the edit's logic: check proj-MM medians (379ns vs ~450ns).
 - Other measured dead ends: PE-side K=1 ones-matmul broadcast replacing
   the GPSIMD broadcast (kills the lib swap and all >1.3us stalls but nets
   +14us: +7.7us PE busy and per-pair micro-stalls); fused [1,2,CH]
   recip/broadcast (serializes the heads' normalize chains, +8us); bf16 y
   output + normalize reorder (+11us, scheduler weave perturbation);
   SBUF->SBUF stride-0-partition DMA broadcast (rejected: "AP partition
   dimension must have nonzero step"); ones-column-first V layout (BIR
   rejects 64-partition reads starting at partition 1).
"""

import numpy as np
import ml_dtypes

BF16 = ml_dtypes.bfloat16

S = 2048
D = 768
NH = 12
H = 64
P = 128
NH_LOC = 6
FEAT = NH_LOC * H          # 384
KO = D // P                # 6 contraction tiles for the projections
FT = FEAT // P             # 3 feature partition-tiles
CH = 512                   # sq chunk width
NCH = S // CH              # 4
ST = S // P                # 16 sequence partition-tiles
KPC = CH // P              # 4 sk-tiles per chunk
N_CORES = 8

_NC_CACHE = {}


def _build_nc():
    import concourse.bass as bass
    import concourse.mybir as mybir
    import concourse.tile as tile
    from concourse import bacc

    f32 = mybir.dt.float32
    bf16 = mybir.dt.bfloat16
    AF = mybir.ActivationFunctionType
    ALU = mybir.AluOpType

    nc = bacc.Bacc("TRN2", target_bir_lowering=False, debug=False)

    xT = nc.dram_tensor("xT", [D, S], bf16, kind="ExternalInput").ap()
    wqkv = nc.dram_tensor("wqkv", [D, 3 * FEAT], bf16, kind="ExternalInput").ap()
    bqkv = nc.dram_tensor("bqkv", [3 * FEAT], f32, kind="ExternalInput").ap()
    wo = nc.dram_tensor("wo", [FEAT, D], bf16, kind="ExternalInput").ap()
    y = nc.dram_tensor("y", [S, D], f32, kind="ExternalOutput").ap()

    with tile.TileContext(nc) as tc:
        with (
            tc.tile_pool(name="persist", bufs=1) as persist,
            tc.tile_pool(name="mm_ps", bufs=2, space="PSUM") as mm_ps,
            tc.tile_pool(name="sc_ps", bufs=2, space="PSUM") as sc_ps,
            tc.tile_pool(name="z_ps", bufs=2, space="PSUM") as z_ps,
            tc.tile_pool(name="p_pool", bufs=10) as p_pool,
            tc.tile_pool(name="n_pool", bufs=6) as n_pool,
            tc.tile_pool(name="y_pool", bufs=3) as y_pool,
        ):
            x_sb = persist.tile([P, KO, S], bf16)
            xT_t = xT.rearrange("(ko pi) s -> pi ko s", pi=P)
            w_sb = persist.tile([P, KO, 3 * FEAT], bf16)
            w_t = wqkv.rearrange("(ko pi) f -> pi ko f", pi=P)
            for ko in range(KO):
                nc.sync.dma_start(
                    out=w_sb[:, ko, 0 : 2 * FEAT], in_=w_t[:, ko, 0 : 2 * FEAT]
                )
                nc.scalar.dma_start(out=x_sb[:, ko, 0:CH], in_=xT_t[:, ko, 0:CH])
            bias_sb = persist.tile([P, 9], f32)
            nc.scalar.dma_start(
                out=bias_sb, in_=bqkv.rearrange("(t pi) -> pi t", pi=P)
            )
            bvb_sb = persist.tile([P, FEAT], f32)
            bv_slice = bqkv[2 * FEAT : 3 * FEAT]
            bv_bcast = bass.AP(
                tensor=bv_slice.tensor,
                offset=bv_slice.offset,
                ap=[[0, P]] + list(bv_slice.ap),
            )
            nc.scalar.dma_start(out=bvb_sb, in_=bv_bcast)
            for ko in range(KO):
                nc.sync.dma_start(
                    out=w_sb[:, ko, 2 * FEAT :], in_=w_t[:, ko, 2 * FEAT :]
                )
            for c in range(1, NCH):
                for ko in range(KO):
                    eng = nc.scalar if (ko % 2) else nc.sync
                    eng.dma_start(
                        out=x_sb[:, ko, c * CH : (c + 1) * CH],
                        in_=xT_t[:, ko, c * CH : (c + 1) * CH],
                    )


            wo_sb = persist.tile([P, FT, D], bf16)
            wo_t = wo.rearrange("(ft pi) d -> pi ft d", pi=P)
            nc.sync.dma_start(out=wo_sb, in_=wo_t)

            qT_sb = persist.tile([P, FT, S], bf16)
            kT_sb = persist.tile([P, FT, S], bf16)
            v_sb = persist.tile([P, ST, NH_LOC, H + 1], bf16)
            zn_sb = persist.tile([P, FT, S], bf16)

            nc.vector.memset(v_sb[:, :, :, H : H + 1], 1.0)

            warm = n_pool.tile([1, 1], f32, tag="warm")
            nc.vector.memset(warm, 0.0)
            nc.scalar.activation(out=warm, in_=warm, func=AF.Exp)

            # trigger the GPSIMD custom-op library load during the initial
            # DMA-wait window: the first partition_broadcast otherwise pays
            # a ~6us UNLOAD/LOAD_LIB queue stall right at the first chunk
            # boundary, which every other engine ends up waiting on
            gwi = n_pool.tile([1, CH], f32, tag="drow", name="gwi")
            gwo = n_pool.tile([H, CH], f32, tag="rb", name="gwo")
            nc.vector.memset(gwi, 1.0)
            nc.gpsimd.partition_broadcast(gwo, gwi)

            wrm_sb = persist.tile([P, CH], bf16)
            nc.vector.memset(wrm_sb, 0.0)
            wrm_ps = mm_ps.tile([P, CH], f32, tag="mm", name="wrm_ps")
            for i in range(14):
                nc.tensor.matmul(
                    wrm_ps,
                    lhsT=wrm_sb[:, 0:P],
                    rhs=wrm_sb,
                    start=(i == 0),
                    stop=(i == 13),
                )

            def proj_pieces(c):
                pieces = []
                for which, base in ((0, 0), (1, FEAT)):
                    sb = qT_sb if which == 0 else kT_sb
                    for ft in range(FT):
                        def qk_piece(which=which, base=base, sb=sb, ft=ft):
                            ps = mm_ps.tile([P, CH], f32, tag="mm", name="proj_ps")
                            for ko in range(KO):
                                nc.tensor.matmul(
                                    ps,
                                    lhsT=w_sb[
                                        :, ko, base + ft * P : base + (ft + 1) * P
                                    ],
                                    rhs=x_sb[:, ko, c * CH : (c + 1) * CH],
                                    start=(ko == 0),
                                    stop=(ko == KO - 1),
                                )
                            nc.vector.tensor_scalar_add(
                                out=sb[:, ft, c * CH : (c + 1) * CH],
                                in0=ps,
                                scalar1=bias_sb[
                                    :, which * FT + ft : which * FT + ft + 1
                                ],
                            )
                        pieces.append(qk_piece)
                for sti in range(KPC):
                    def v_piece(sti=sti):
                        st = c * KPC + sti
                        ps = mm_ps.tile([P, FEAT], f32, tag="mm", name="projv_ps")
                        for ko in range(KO):
                            nc.tensor.matmul(
                                ps,
                                lhsT=x_sb[:, ko, st * P : (st + 1) * P],
                                rhs=w_sb[:, ko, 2 * FEAT : 3 * FEAT],
                                start=(ko == 0),
                                stop=(ko == KO - 1),
                            )
                        nc.vector.tensor_add(
                            out=v_sb[:, st, :, 0:H],
                            in0=ps.rearrange("p (n h) -> p n h", h=H),
                            in1=bvb_sb.rearrange("p (n h) -> p n h", h=H),
                        )
                    pieces.append(v_piece)
                return pieces

            def attn_pair(q, hp, fillers=(), tail_warm=False):
                nkt = (q + 1) * KPC
                if True:
                    ft = hp
                    h0, h1 = 2 * hp, 2 * hp + 1
                    zts = [
                        z_ps.tile([H + 1, CH], f32, tag="z", name="zt0"),
                        z_ps.tile([H + 1, CH], f32, tag="z", name="zt1"),
                    ]
                    for kt in range(nkt):
                        c = max(0, kt * P - q * CH)
                        diag = kt >= q * KPC
                        pg = sc_ps.tile([P, 2, CH], f32, tag="sc", name="sc_ps_t")
                        for hi, r0 in ((0, 0), (1, H)):
                            nc.tensor.matmul(
                                pg[:, hi, c:],
                                lhsT=kT_sb[r0 : r0 + H, ft, kt * P : (kt + 1) * P],
                                rhs=qT_sb[r0 : r0 + H, ft, q * CH + c : (q + 1) * CH],
                                start=True,
                                stop=True,
                            )
                        pt = p_pool.tile([P, 2, CH], bf16, tag="pt", name="pt_t")
                        nc.scalar.activation(
                            out=pt[:, :, c:], in_=pg[:, :, c:], func=AF.Exp
                        )
                        if diag:
                            nc.gpsimd.affine_select(
                                out=pt[:, :, c:],
                                in_=pt[:, :, c:],
                                compare_op=ALU.is_ge,
                                fill=0.0,
                                base=0,
                                pattern=[[0, 2], [1, CH - c]],
                                channel_multiplier=-1,
                            )
                        for hi, h in ((0, h0), (1, h1)):
                            nc.tensor.matmul(
                                zts[hi][:, c:],
                                lhsT=v_sb[:, kt, h, :],
                                rhs=pt[:, hi, c:],
                                start=(kt == 0),
                                stop=(kt == nkt - 1),
                            )
                    zsbs = []
                    for hi in (0, 1):
                        zsb = n_pool.tile([H + 1, CH], f32, tag="zsb", name="zsb_t")
                        nc.vector.tensor_copy(out=zsb, in_=zts[hi])
                        zsbs.append(zsb)
                    if tail_warm:
                        # dummy matmuls pinned to the tail (they consume the
                        # last pt tile so the scheduler cannot hoist them):
                        # keep the PE at the warm HAM clock through the final
                        # normalize chain so the last output projection does
                        # not run at the cold 1.2 GHz clock
                        wrm2_ps = mm_ps.tile([P, CH], f32, tag="mm", name="wrm2_ps")
                        for i in range(8):
                            nc.tensor.matmul(
                                wrm2_ps,
                                lhsT=wrm_sb[:, 0:P],
                                rhs=pt[:, 0, :],
                                start=(i == 0),
                                stop=(i == 7),
                            )
                    for f in fillers:
                        f()
                    rbs = []
                    for hi in (0, 1):
                        drow = n_pool.tile([1, CH], f32, tag="drow", name="drow_t")
                        nc.vector.tensor_copy(out=drow, in_=zsbs[hi][H : H + 1, :])
                        rrow = n_pool.tile([1, CH], f32, tag="rrow", name="rrow_t")
                        nc.vector.reciprocal_approx_fast(out=rrow, in_=drow)
                        rb = n_pool.tile([H, CH], f32, tag="rb", name="rb_t")
                        nc.gpsimd.partition_broadcast(rb, rrow)
                        rbs.append(rb)
                    for hi, r0 in ((0, 0), (1, H)):
                        nc.vector.tensor_mul(
                            out=zn_sb[r0 : r0 + H, ft, q * CH : (q + 1) * CH],
                            in0=zsbs[hi][0:H, :],
                            in1=rbs[hi],
                        )

            def out_pieces(q):
                pieces = []
                for sti in range(KPC):
                    def out_piece(sti=sti):
                        st = q * KPC + sti
                        ysb = y_pool.tile([P, D], f32, tag="y", name="y_t")
                        psa = mm_ps.tile([P, D // 2], f32, tag="mm", name="yps_a")
                        psb = mm_ps.tile([P, D // 2], f32, tag="mm", name="yps_b")
                        for ft in range(FT):
                            for ps, dh in ((psa, 0), (psb, 1)):
                                nc.tensor.matmul(
                                    ps,
                                    lhsT=zn_sb[:, ft, st * P : (st + 1) * P],
                                    rhs=wo_sb[
                                        :, ft, dh * (D // 2) : (dh + 1) * (D // 2)
                                    ],
                                    start=(ft == 0),
                                    stop=(ft == FT - 1),
                                )
                        for ps, dh in ((psa, 0), (psb, 1)):
                            nc.vector.tensor_copy(
                                out=ysb[:, dh * (D // 2) : (dh + 1) * (D // 2)],
                                in_=ps,
                            )
                        nc.sync.dma_start(out=y[st * P : (st + 1) * P, :], in_=ysb)
                    pieces.append(out_piece)
                return pieces

            for piece in proj_pieces(0):
                piece()
            for q in range(NCH):
                fillers = []
                if q + 1 < NCH:
                    fillers += proj_pieces(q + 1)
                if q >= 1:
                    fillers += out_pieces(q - 1)
                npairs = NH_LOC // 2
                fi = 0
                for hp in range(npairs):
                    share = ((hp + 1) * len(fillers)) // npairs
                    attn_pair(
                        q,
                        hp,
                        fillers[fi:share],
                        tail_warm=(q == NCH - 1 and hp == npairs - 1),
                    )
                    fi = share
            for piece in out_pieces(NCH - 1):
                piece()

    nc.compile()
    return nc


def _get_nc():
    if "nc" not in _NC_CACHE:
        _NC_CACHE["nc"] = _build_nc()
    return _NC_CACHE["nc"]


def _shard_inputs(x, W_Q, W_K, W_V, W_O, b_Q, b_K, b_V):
    in_maps = []
    scale = np.float32(1.0 / np.sqrt(H))
    for c in range(N_CORES):
        b = c % 4
        g = c // 4
        hs = slice(g * NH_LOC, (g + 1) * NH_LOC)
        xTb = np.ascontiguousarray(x[b].T).astype(BF16)
        wq = (W_Q[hs].transpose(1, 0, 2).reshape(D, FEAT) * scale).astype(BF16)
        wk = W_K[hs].transpose(1, 0, 2).reshape(D, FEAT).astype(BF16)
        wv = W_V[hs].transpose(1, 0, 2).reshape(D, FEAT).astype(BF16)
        wqkv = np.ascontiguousarray(np.concatenate([wq, wk, wv], axis=1))
        bqkv = np.concatenate(
            [
                (b_Q[hs].reshape(FEAT) * scale).astype(np.float32),
                b_K[hs].reshape(FEAT).astype(np.float32),
                b_V[hs].reshape(FEAT).astype(np.float32),
            ]
        )
        wob = np.ascontiguousarray(W_O[hs].reshape(FEAT, D)).astype(BF16)
        in_maps.append({"xT": xTb, "wqkv": wqkv, "bqkv": bqkv, "wo": wob})
    return in_maps


def _run(in_maps, trace=False):
    from concourse.bass_utils import run_bass_kernel_spmd

    nc = _get_nc()
    return run_bass_kernel_spmd(nc, in_maps, core_ids=list(range(N_CORES)), trace=trace)


def kernel(
    normalized_resid_pre,
    W_Q,
    W_K,
    W_V,
    W_O,
    b_Q,
    b_K,
    b_V,
    b_O,
):
    x = np.asarray(normalized_resid_pre, dtype=np.float32)
    in_maps = _shard_inputs(
        x,
        np.asarray(W_Q, np.float32),
        np.asarray(W_K, np.float32),
        np.asarray(W_V, np.float32),
        np.asarray(W_O, np.float32),
        np.asarray(b_Q, np.float32),
        np.asarray(b_K, np.float32),
        np.asarray(b_V, np.float32),
    )
    res = _run(in_maps)
    bO = np.asarray(b_O, np.float32)
    out = np.empty((4, S, D), dtype=np.float32)
    for b in range(4):
        out[b] = res.results[b]["y"] + res.results[4 + b]["y"] + bO
    return out


# revision 26
# speedup vs baseline: 1.2734x; 1.0002x over previous
"""Trainium2 Bass kernel for causal multi-head attention (GPT2-ish block).

Full-input contract: kernel(**inputs) takes the unsharded inputs of
  x: [4, 2048, 768], W_Q/W_K/W_V: [12, 768, 64], W_O: [12, 64, 768],
  b_Q/b_K/b_V: [12, 64], b_O: [768]
and returns out: [4, 2048, 768] fp32.

Sharding: 8 cores = 4 batches x 2 head-groups (6 heads each); each core
computes its batch's attention for its 6 heads through W_O (partial sum over
heads). Host sums the two head-group partials per batch and adds b_O.

Per-core layouts (all matmuls are out = lhsT.T @ rhs on the PE):
  xT   [768, 2048]      (d_model on partitions)
  QT,KT[384, 2048]      (head-features on partitions)  q pre-scaled by 1/8
  V    [2048, 384]      (+ ones column per head for softmax denominators)
  scoresT tiles [sk=128, sq<=512] -> exp on ACT -> causal zero on GPSIMD
  zT'  [65, 512]        rows 0-63 unnormalized z, row 64 = softmax denom
  y    [2048, 768] fp32

Structure notes (see git-less history in the perf notes below):
 - Heads are processed in pairs on partition rows 0:64 / 64:128 of one
   feature tile; their K=64 scores matmuls go to different PE row groups and
   run concurrently in the array (measured dstart ~4ns).
 - A pair's two scores tiles share one [128, 2, 512] two-bank psum group so
   a single ACTIVATE exps both heads (amortizes the ~350-cycle ACT setup).
 - Diagonal tiles are trimmed to their valid sq suffix; causal zeroing on
   GPSIMD (affine_select).
 - Emission is woven: the independent PE work of proj(q+1) and out(q-1) is
   emitted inside attention chunk q so the PE never starves during exps.
 - Softmax denominator rides as a ones-column in the AV stationary ([V | 1]);
   normalization = custom-DVE reciprocal (reads partition 0; denominator row
   staged there) + GPSIMD partition broadcast + DVE multiply.
 - A dummy partition_broadcast right after the ACT exp-table warm triggers
   the GPSIMD custom-op library UNLOAD/LOAD_LIB during the initial DMA-wait
   window; without it the first real broadcast pays a ~6us GPSIMD-queue
   stall at the first chunk boundary that every engine convoys behind
   (measured: 212.1 -> 206.1us).
 - The last pair of the last chunk emits 8 dummy matmuls that consume its pt
   tile (so the scheduler cannot hoist them): they bridge the PE-idle window
   during the final normalize chain so the HAM clock gate stays at 8/8 and
   the last output projection runs at 2.4 GHz, not 1.2 (214.6 -> 212.1us).

Perf notes from trace analysis (best measured: 206124 ns):
 - Engine busy at 206us: Tensor ~171us (largest idle gap 1.3us), ACT (exp)
   ~124us, DVE ~102us, GPSIMD ~62us. ACT floor for the 120 exp ACTIVATEs is
   (free_elems + 352*n)/1.2GHz ~= 122us; batching bigger ACTIVATEs needs >8
   psum banks, unreachable at CH=512.
 - CLOCK TRAP: some NEFF layouts of this kernel run every engine at exactly
   5/6 clock for the whole execution (proj MM 379ns -> ~451ns, exec * 1.20),
   reproducibly for that binary while other builds interleaved on the same
   device stay fast. Observed on: (a) a variant adding a degenerate
   affine_select+broadcast warm-up pair, (b) a variant adding a Q/K-split
   weight-DMA reorder on top of this exact file. The current file measured
   full-clock. If a future edit regresses ~20% uniformly, suspect this
   before blaming the edit's logic: check proj-MM medians (379ns vs ~450ns).
 - Other measured dead ends: PE-side K=1 ones-matmul broadcast replacing
   the GPSIMD broadcast (kills the lib swap and all >1.3us stalls but nets
   +14us: +7.7us PE busy and per-pair micro-stalls); fused [1,2,CH]
   recip/broadcast (serializes the heads' normalize chains, +8us); bf16 y
   output + normalize reorder (+11us, scheduler weave perturbation);
   SBUF->SBUF stride-0-partition DMA broadcast (rejected: "AP partition
   dimension must have nonzero step"); ones-column-first V layout (BIR
   rejects 64-partition reads starting at partition 1).
 - BIG dead end (the "Round 2" repack, measured 278.9us at full clock,
   numerically correct): scores split into four 64x64-stationary quadrant
   matmuls + the pair's two AV matmuls packed onto disjoint column groups
   of one [128,CH] z bank + denominators via two ones-stationary matmuls
   (replacing the V ones-column). The hope was cell-level (32x32-grid)
   concurrency like the measured row-group score pairs; in practice the
   extra ~480 matmuls SERIALIZED (MATMUL busy 300us -> 486us). The 4ns
   concurrent-issue behavior does not emerge for this mixed row+col packing
   under the tile scheduler; only the simple two-head row-split (scores)
   reliably overlaps. Note for DVE ops: TensorTensor requires EQUAL base
   partitions when both inputs are SBUF (copies don't).
"""

import numpy as np
import ml_dtypes

BF16 = ml_dtypes.bfloat16

S = 2048
D = 768
NH = 12
H = 64
P = 128
NH_LOC = 6
FEAT = NH_LOC * H          # 384
KO = D // P                # 6 contraction tiles for the projections
FT = FEAT // P             # 3 feature partition-tiles
CH = 512                   # sq chunk width
NCH = S // CH              # 4
ST = S // P                # 16 sequence partition-tiles
KPC = CH // P              # 4 sk-tiles per chunk
N_CORES = 8

_NC_CACHE = {}


def _build_nc():
    import concourse.bass as bass
    import concourse.mybir as mybir
    import concourse.tile as tile
    from concourse import bacc

    f32 = mybir.dt.float32
    bf16 = mybir.dt.bfloat16
    AF = mybir.ActivationFunctionType
    ALU = mybir.AluOpType

    nc = bacc.Bacc("TRN2", target_bir_lowering=False, debug=False)

    xT = nc.dram_tensor("xT", [D, S], bf16, kind="ExternalInput").ap()
    wqkv = nc.dram_tensor("wqkv", [D, 3 * FEAT], bf16, kind="ExternalInput").ap()
    bqkv = nc.dram_tensor("bqkv", [3 * FEAT], f32, kind="ExternalInput").ap()
    wo = nc.dram_tensor("wo", [FEAT, D], bf16, kind="ExternalInput").ap()
    y = nc.dram_tensor("y", [S, D], f32, kind="ExternalOutput").ap()

    with tile.TileContext(nc) as tc:
        with (
            tc.tile_pool(name="persist", bufs=1) as persist,
            tc.tile_pool(name="mm_ps", bufs=2, space="PSUM") as mm_ps,
            tc.tile_pool(name="sc_ps", bufs=2, space="PSUM") as sc_ps,
            tc.tile_pool(name="z_ps", bufs=2, space="PSUM") as z_ps,
            tc.tile_pool(name="p_pool", bufs=10) as p_pool,
            tc.tile_pool(name="n_pool", bufs=6) as n_pool,
            tc.tile_pool(name="y_pool", bufs=3) as y_pool,
        ):
            x_sb = persist.tile([P, KO, S], bf16)
            xT_t = xT.rearrange("(ko pi) s -> pi ko s", pi=P)
            w_sb = persist.tile([P, KO, 3 * FEAT], bf16)
            w_t = wqkv.rearrange("(ko pi) f -> pi ko f", pi=P)
            for ko in range(KO):
                nc.sync.dma_start(
                    out=w_sb[:, ko, 0 : 2 * FEAT], in_=w_t[:, ko, 0 : 2 * FEAT]
                )
                nc.scalar.dma_start(out=x_sb[:, ko, 0:CH], in_=xT_t[:, ko, 0:CH])
            bias_sb = persist.tile([P, 9], f32)
            nc.scalar.dma_start(
                out=bias_sb, in_=bqkv.rearrange("(t pi) -> pi t", pi=P)
            )
            bvb_sb = persist.tile([P, FEAT], f32)
            bv_slice = bqkv[2 * FEAT : 3 * FEAT]
            bv_bcast = bass.AP(
                tensor=bv_slice.tensor,
                offset=bv_slice.offset,
                ap=[[0, P]] + list(bv_slice.ap),
            )
            nc.scalar.dma_start(out=bvb_sb, in_=bv_bcast)
            for ko in range(KO):
                nc.sync.dma_start(
                    out=w_sb[:, ko, 2 * FEAT :], in_=w_t[:, ko, 2 * FEAT :]
                )
            for c in range(1, NCH):
                for ko in range(KO):
                    eng = nc.scalar if (ko % 2) else nc.sync
                    eng.dma_start(
                        out=x_sb[:, ko, c * CH : (c + 1) * CH],
                        in_=xT_t[:, ko, c * CH : (c + 1) * CH],
                    )


            wo_sb = persist.tile([P, FT, D], bf16)
            wo_t = wo.rearrange("(ft pi) d -> pi ft d", pi=P)
            nc.sync.dma_start(out=wo_sb, in_=wo_t)

            qT_sb = persist.tile([P, FT, S], bf16)
            kT_sb = persist.tile([P, FT, S], bf16)
            v_sb = persist.tile([P, ST, NH_LOC, H + 1], bf16)
            zn_sb = persist.tile([P, FT, S], bf16)

            nc.vector.memset(v_sb[:, :, :, H : H + 1], 1.0)

            warm = n_pool.tile([1, 1], f32, tag="warm")
            nc.vector.memset(warm, 0.0)
            nc.scalar.activation(out=warm, in_=warm, func=AF.Exp)

            # trigger the GPSIMD custom-op library load during the initial
            # DMA-wait window: the first partition_broadcast otherwise pays
            # a ~6us UNLOAD/LOAD_LIB queue stall right at the first chunk
            # boundary, which every other engine ends up waiting on
            gwi = n_pool.tile([1, CH], f32, tag="drow", name="gwi")
            gwo = n_pool.tile([H, CH], f32, tag="rb", name="gwo")
            nc.vector.memset(gwi, 1.0)
            nc.gpsimd.partition_broadcast(gwo, gwi)

            wrm_sb = persist.tile([P, CH], bf16)
            nc.vector.memset(wrm_sb, 0.0)
            wrm_ps = mm_ps.tile([P, CH], f32, tag="mm", name="wrm_ps")
            for i in range(14):
                nc.tensor.matmul(
                    wrm_ps,
                    lhsT=wrm_sb[:, 0:P],
                    rhs=wrm_sb,
                    start=(i == 0),
                    stop=(i == 13),
                )

            def proj_pieces(c):
                pieces = []
                for which, base in ((0, 0), (1, FEAT)):
                    sb = qT_sb if which == 0 else kT_sb
                    for ft in range(FT):
                        def qk_piece(which=which, base=base, sb=sb, ft=ft):
                            ps = mm_ps.tile([P, CH], f32, tag="mm", name="proj_ps")
                            for ko in range(KO):
                                nc.tensor.matmul(
                                    ps,
                                    lhsT=w_sb[
                                        :, ko, base + ft * P : base + (ft + 1) * P
                                    ],
                                    rhs=x_sb[:, ko, c * CH : (c + 1) * CH],
                                    start=(ko == 0),
                                    stop=(ko == KO - 1),
                                )
                            nc.vector.tensor_scalar_add(
                                out=sb[:, ft, c * CH : (c + 1) * CH],
                                in0=ps,
                                scalar1=bias_sb[
                                    :, which * FT + ft : which * FT + ft + 1
                                ],
                            )
                        pieces.append(qk_piece)
                for sti in range(KPC):
                    def v_piece(sti=sti):
                        st = c * KPC + sti
                        ps = mm_ps.tile([P, FEAT], f32, tag="mm", name="projv_ps")
                        for ko in range(KO):
                            nc.tensor.matmul(
                                ps,
                                lhsT=x_sb[:, ko, st * P : (st + 1) * P],
                                rhs=w_sb[:, ko, 2 * FEAT : 3 * FEAT],
                                start=(ko == 0),
                                stop=(ko == KO - 1),
                            )
                        nc.vector.tensor_add(
                            out=v_sb[:, st, :, 0:H],
                            in0=ps.rearrange("p (n h) -> p n h", h=H),
                            in1=bvb_sb.rearrange("p (n h) -> p n h", h=H),
                        )
                    pieces.append(v_piece)
                return pieces

            def attn_pair(q, hp, fillers=(), tail_warm=False):
                nkt = (q + 1) * KPC
                if True:
                    ft = hp
                    h0, h1 = 2 * hp, 2 * hp + 1
                    zts = [
                        z_ps.tile([H + 1, CH], f32, tag="z", name="zt0"),
                        z_ps.tile([H + 1, CH], f32, tag="z", name="zt1"),
                    ]
                    for kt in range(nkt):
                        c = max(0, kt * P - q * CH)
                        diag = kt >= q * KPC
                        pg = sc_ps.tile([P, 2, CH], f32, tag="sc", name="sc_ps_t")
                        for hi, r0 in ((0, 0), (1, H)):
                            nc.tensor.matmul(
                                pg[:, hi, c:],
                                lhsT=kT_sb[r0 : r0 + H, ft, kt * P : (kt + 1) * P],
                                rhs=qT_sb[r0 : r0 + H, ft, q * CH + c : (q + 1) * CH],
                                start=True,
                                stop=True,
                            )
                        pt = p_pool.tile([P, 2, CH], bf16, tag="pt", name="pt_t")
                        nc.scalar.activation(
                            out=pt[:, :, c:], in_=pg[:, :, c:], func=AF.Exp
                        )
                        if diag:
                            nc.gpsimd.affine_select(
                                out=pt[:, :, c:],
                                in_=pt[:, :, c:],
                                compare_op=ALU.is_ge,
                                fill=0.0,
                                base=0,
                                pattern=[[0, 2], [1, CH - c]],
                                channel_multiplier=-1,
                            )
                        for hi, h in ((0, h0), (1, h1)):
                            nc.tensor.matmul(
                                zts[hi][:, c:],
                                lhsT=v_sb[:, kt, h, :],
                                rhs=pt[:, hi, c:],
                                start=(kt == 0),
                                stop=(kt == nkt - 1),
                            )
                    zsbs = []
                    for hi in (0, 1):
                        zsb = n_pool.tile([H + 1, CH], f32, tag="zsb", name="zsb_t")
                        nc.vector.tensor_copy(out=zsb, in_=zts[hi])
                        zsbs.append(zsb)
                    if tail_warm:
                        # dummy matmuls pinned to the tail (they consume the
                        # last pt tile so the scheduler cannot hoist them):
                        # keep the PE at the warm HAM clock through the final
                        # normalize chain so the last output projection does
                        # not run at the cold 1.2 GHz clock
                        wrm2_ps = mm_ps.tile([P, CH], f32, tag="mm", name="wrm2_ps")
                        for i in range(8):
                            nc.tensor.matmul(
                                wrm2_ps,
                                lhsT=wrm_sb[:, 0:P],
                                rhs=pt[:, 0, :],
                                start=(i == 0),
                                stop=(i == 7),
                            )
                    for f in fillers:
                        f()
                    rbs = []
                    for hi in (0, 1):
                        drow = n_pool.tile([1, CH], f32, tag="drow", name="drow_t")
                        nc.vector.tensor_copy(out=drow, in_=zsbs[hi][H : H + 1, :])
                        rrow = n_pool.tile([1, CH], f32, tag="rrow", name="rrow_t")
                        nc.vector.reciprocal_approx_fast(out=rrow, in_=drow)
                        rb = n_pool.tile([H, CH], f32, tag="rb", name="rb_t")
                        nc.gpsimd.partition_broadcast(rb, rrow)
                        rbs.append(rb)
                    for hi, r0 in ((0, 0), (1, H)):
                        nc.vector.tensor_mul(
                            out=zn_sb[r0 : r0 + H, ft, q * CH : (q + 1) * CH],
                            in0=zsbs[hi][0:H, :],
                            in1=rbs[hi],
                        )

            def out_pieces(q):
                pieces = []
                for sti in range(KPC):
                    def out_piece(sti=sti):
                        st = q * KPC + sti
                        ysb = y_pool.tile([P, D], f32, tag="y", name="y_t")
                        psa = mm_ps.tile([P, D // 2], f32, tag="mm", name="yps_a")
                        psb = mm_ps.tile([P, D // 2], f32, tag="mm", name="yps_b")
                        for ft in range(FT):
                            for ps, dh in ((psa, 0), (psb, 1)):
                                nc.tensor.matmul(
                                    ps,
                                    lhsT=zn_sb[:, ft, st * P : (st + 1) * P],
                                    rhs=wo_sb[
                                        :, ft, dh * (D // 2) : (dh + 1) * (D // 2)
                                    ],
                                    start=(ft == 0),
                                    stop=(ft == FT - 1),
                                )
                        for ps, dh in ((psa, 0), (psb, 1)):
                            nc.vector.tensor_copy(
                                out=ysb[:, dh * (D // 2) : (dh + 1) * (D // 2)],
                                in_=ps,
                            )
                        nc.sync.dma_start(out=y[st * P : (st + 1) * P, :], in_=ysb)
                    pieces.append(out_piece)
                return pieces

            for piece in proj_pieces(0):
                piece()
            deferred = []
            for q in range(NCH):
                fillers = []
                if q + 1 < NCH:
                    pp = proj_pieces(q + 1)
                    if q == NCH - 2:
                        # chunk 3 is ACT-bound (48.9us of exp vs ~41us of PE
                        # work) while chunk 2 is PE-bound: defer chunk 3's
                        # ft1/ft2 Q/K projection pieces out of chunk 2's
                        # fillers into chunk 3's own — pair hp's fillers
                        # produce ft hp+1 just before pair hp+1 needs it
                        deferred = [pp[1], pp[4], pp[2], pp[5]]
                        pp = [pp[0], pp[3]] + pp[6:]
                    fillers += pp
                if q == NCH - 1:
                    fillers = deferred + fillers
                if q >= 1:
                    fillers += out_pieces(q - 1)
                npairs = NH_LOC // 2
                fi = 0
                for hp in range(npairs):
                    share = ((hp + 1) * len(fillers)) // npairs
                    attn_pair(
                        q,
                        hp,
                        fillers[fi:share],
                        tail_warm=(q == NCH - 1 and hp == npairs - 1),
                    )
                    fi = share
            for piece in out_pieces(NCH - 1):
                piece()

    nc.compile()
    return nc


def _get_nc():
    if "nc" not in _NC_CACHE:
        _NC_CACHE["nc"] = _build_nc()
    return _NC_CACHE["nc"]


def _shard_inputs(x, W_Q, W_K, W_V, W_O, b_Q, b_K, b_V):
    in_maps = []
    scale = np.float32(1.0 / np.sqrt(H))
    for c in range(N_CORES):
        b = c % 4
        g = c // 4
        hs = slice(g * NH_LOC, (g + 1) * NH_LOC)
        xTb = np.ascontiguousarray(x[b].T).astype(BF16)
        wq = (W_Q[hs].transpose(1, 0, 2).reshape(D, FEAT) * scale).astype(BF16)
        wk = W_K[hs].transpose(1, 0, 2).reshape(D, FEAT).astype(BF16)
        wv = W_V[hs].transpose(1, 0, 2).reshape(D, FEAT).astype(BF16)
        wqkv = np.ascontiguousarray(np.concatenate([wq, wk, wv], axis=1))
        bqkv = np.concatenate(
            [
                (b_Q[hs].reshape(FEAT) * scale).astype(np.float32),
                b_K[hs].reshape(FEAT).astype(np.float32),
                b_V[hs].reshape(FEAT).astype(np.float32),
            ]
        )
        wob = np.ascontiguousarray(W_O[hs].reshape(FEAT, D)).astype(BF16)
        in_maps.append({"xT": xTb, "wqkv": wqkv, "bqkv": bqkv, "wo": wob})
    return in_maps


def _run(in_maps, trace=False):
    from concourse.bass_utils import run_bass_kernel_spmd

    nc = _get_nc()
    return run_bass_kernel_spmd(nc, in_maps, core_ids=list(range(N_CORES)), trace=trace)


def kernel(
    normalized_resid_pre,
    W_Q,
    W_K,
    W_V,
    W_O,
    b_Q,
    b_K,
    b_V,
    b_O,
):
    x = np.asarray(normalized_resid_pre, dtype=np.float32)
    in_maps = _shard_inputs(
        x,
        np.asarray(W_Q, np.float32),
        np.asarray(W_K, np.float32),
        np.asarray(W_V, np.float32),
        np.asarray(W_O, np.float32),
        np.asarray(b_Q, np.float32),
        np.asarray(b_K, np.float32),
        np.asarray(b_V, np.float32),
    )
    res = _run(in_maps)
    bO = np.asarray(b_O, np.float32)
    out = np.empty((4, S, D), dtype=np.float32)
    for b in range(4):
        out[b] = res.results[b]["y"] + res.results[4 + b]["y"] + bO
    return out
